# revision 42
# baseline (speedup 1.0000x reference)
"""Trainium2 Bass kernel for nn_AEFIN (FFT top-k masking + attention + FAN/MLP).

Data-parallel over batch: 64 batches sharded 8-per-core across 8 NeuronCores.
Inside each core (all shapes hardcoded for BS=64, L=512, E=64, pred=512):

  per pair of batches (c_pair = 2*64 = 128 channels packed on partitions):
    rfft as f32 matmul (exact enough for top-k selection)   [PE, f32]
    mag^2 -> 8-at-a-time max + match_replace top-k zap      [DVE]
    mask = (zap==0); masked spectra in bf16                 [DVE]
    irfft as bf16 matmul -> x_filt (both layouts), norm     [PE]
  per batch: single-head attention with host-folded weights [PE/ACT, bf16]
  core-wide: FAN (sin/cos/gelu) + 2-layer MLP               [PE/ACT, bf16]

Schedule: the x-half of the MLP's first layer (z1x = fc1[:,512:] @ x) depends
only on the input and fc1 weights, so its 96 matmuls are interleaved into the
FFT/top-k phase (which otherwise idles the PE while the DVE runs top-k rounds
and lets the HAM clock gate re-throttle to 1.2 GHz).  z1x chunks stash to the
z1r tiles in bf16; the fan-half accumulates later and a fused DVE
scalar_tensor_tensor adds stash + bias before the relu.

Host-side folds: attention score matrix Wm = (wq/8)^T wk, k-bias folded as a
per-partition bias on the ttbf drain (tt' = tt + r adds r.n_key to every
score column == the additive per-key bias), v/out_proj merged (wvo = out_w @
wv, bias folded through attn row-sum), FAN gate sigmoid folded into fc1
columns, all weight transposes done on host.
"""

import math
import os
import sys

for _p in ("/opt/trn_rl_repo",):
    if _p not in sys.path and os.path.isdir(_p):
        sys.path.append(_p)

import numpy as np
import ml_dtypes

BF = ml_dtypes.bfloat16
SEQ, PRED, E, BS = 512, 512, 64, 64
F = SEQ // 2 + 1  # 257
FP = F + 1        # 258: fp32r matmuls need an even moving size
NCORES = 8
BPC = BS // NCORES       # batches per core = 8
NPAIR = BPC // 2         # 4


def _host_dft():
    n = np.arange(SEQ, dtype=np.float64)
    f = np.arange(F, dtype=np.float64)
    ang = 2.0 * np.pi * np.outer(n, f) / SEQ            # [512, 257]
    CrT = np.cos(ang).astype(np.float32)
    SiT = (-np.sin(ang)).astype(np.float32)
    w = np.full(F, 2.0 / SEQ)
    w[0] = 1.0 / SEQ
    w[-1] = 1.0 / SEQ
    angT = ang.T                                        # [257, 512]
    ArT = (w[:, None] * np.cos(angT)).astype(BF)
    AiT = (-(w[:, None]) * np.sin(angT)).astype(BF)
    return CrT, SiT, ArT, AiT


def _mk_layout(entries):
    off, c = {}, 0
    for name, w in entries:
        off[name] = (c, c + w)
        c += w
    return off, c


# f32 constant blob columns: f32 identity + biases + attn k-bias column
FOFF, FW = _mk_layout([
    ("idf", 128), ("bpc", 1), ("bps", 1), ("bg", 2), ("b1", 24), ("b2", 4),
    ("rc", 1),
])
# bf16 constant blob columns. First section (idb/art/ait) is needed by the
# FFT phase and is DMA'd first; the rest arrives later.
BOFF, BW = _mk_layout([
    ("idb", 128), ("art", 3 * SEQ), ("ait", 3 * SEQ),
    ("wm", 128), ("wvo", 130), ("wpt", 4 * 128), ("wgt", 4 * 256),
    ("vb", 130), ("one", 128),
])
BCRIT = BOFF["ait"][1]   # end of the FFT-critical section

_BUILD_CACHE = {}


def _build(k):
    """Build the SPMD Bass graph (identical on all cores). Returns nc."""
    import concourse.mybir as mybir
    import concourse.tile as tile
    from concourse import bacc
    from contextlib import ExitStack

    f32 = mybir.dt.float32
    f32r = mybir.dt.float32r
    bf16 = mybir.dt.bfloat16
    AF = mybir.ActivationFunctionType
    ALU = mybir.AluOpType

    nc = bacc.Bacc("TRN2", target_bir_lowering=False)

    # ---- DRAM parameters (per-core) ----
    P = {}

    def dparam(name, shape, dt):
        P[name] = nc.declare_dram_parameter(name, list(shape), dt, isOutput=False)
        return P[name]

    x_d = dparam("x", [BPC, SEQ, E], f32r)
    blobf_d = dparam("blobf", [128, FW], f32)
    blobb_d = dparam("blobb", [128, BW], bf16)
    fc1_d = dparam("fc1t", [1024, 3072], bf16)
    fc2_d = dparam("fc2t", [3072, 512], bf16)
    out_d = nc.declare_dram_parameter("out", [2, BPC, SEQ, E], f32, isOutput=True)

    rounds = []
    rem = k
    while rem > 0:
        rounds.append(min(8, rem))
        rem -= 8

    with tile.TileContext(nc) as tc, ExitStack() as ctx:
        sb = ctx.enter_context(tc.tile_pool(name="sb", bufs=1))
        ps = ctx.enter_context(tc.tile_pool(name="ps", bufs=1, space="PSUM"))

        def st(shape, dt, tag, bufs=1):
            return sb.tile(shape, dt, tag=tag, bufs=bufs, name=tag)

        def pt(shape, tag, bufs, dt=None):
            return ps.tile(shape, dt or mybir.dt.float32, tag=tag, bufs=bufs,
                           name=tag)

        dma = nc.sync.dma_start
        gdma = nc.gpsimd.dma_start

        # ---- SBUF tiles for constants ----
        # crtb/sitb (the forward DFT) are GENERATED on-chip during the DMA
        # head: nf = n x f (rank-1 f32r matmul, exact), m = nf mod 512,
        # sit = -sin(2pi n f/512) = sin(m*pi/256 - pi),
        # crt =  cos(2pi n f/512) = 2*sin(m*pi/512 - pi/2)^2 - 1.
        crtb = st([128, 4 * FP], f32r, "crtb")
        sitb = st([128, 4 * FP], f32r, "sitb")
        blobf = st([128, FW], f32, "blobf")
        blobb = st([128, BW], bf16, "blobb")

        def fview(lo, hi):
            return blobf[:, lo:hi]

        crt = crtb.rearrange("p (a f) -> p a f", f=FP)
        sit = sitb.rearrange("p (a f) -> p a f", f=FP)
        identf = fview(*FOFF["idf"])
        bpcs = fview(*FOFF["bpc"])
        bpss = fview(*FOFF["bps"])
        bg2 = fview(*FOFF["bg"])
        b1 = fview(*FOFF["b1"])
        b2 = fview(*FOFF["b2"])
        rcol = fview(*FOFF["rc"])

        def bview(lo, hi):
            return blobb[:, lo:hi]

        wm = bview(*BOFF["wm"])
        wvo = bview(*BOFF["wvo"])
        identb = bview(*BOFF["idb"])
        wpt = bview(*BOFF["wpt"]).rearrange("p (a m) -> p a m", m=128)
        wgt = bview(*BOFF["wgt"]).rearrange("p (a m) -> p a m", m=256)
        art = bview(*BOFF["art"]).rearrange("p (a m) -> p a m", m=SEQ)
        ait = bview(*BOFF["ait"]).rearrange("p (a m) -> p a m", m=SEQ)
        vbias = blobb[0:1, BOFF["vb"][0]:BOFF["vb"][1]]
        ones1 = blobb[0:1, BOFF["one"][0]:BOFF["one"][1]]

        wup = st([128, 640], bf16, "wup")
        nc.vector.memset(wup, 1.0)

        # per-pair input x: [part p, batch j, (l c)] with partition p holding
        # time rows 4p..4p+3 contiguously -> 1 KiB DMA descriptors. The
        # interleaved-L semantics propagate to all L-indexed constants
        # (host-permuted, chunk a = rows a::4).
        xps = [st([128, 2, 256], f32r, "xp", bufs=NPAIR) for _ in range(NPAIR)]
        # on-chip reshuffle to [p, l, (j c)]: the 2D-contiguous DMA layout
        # keeps descriptors at 1 KiB; the copies stay f32r end-to-end (the
        # BIR verifier requires f32r-rounded producers for f32r matmuls)
        xqs = [st([128, 4, 128], f32r, "xq", bufs=NPAIR) for _ in range(NPAIR)]

        def xq_fill(p, eng):
            for l in range(4):
                eng(xqs[p][:, l, :].rearrange("p (j c) -> p j c", j=2),
                    xps[p][:, :, l * 64:(l + 1) * 64])
        fc1 = st([128, 8, 3072], bf16, "fc1")
        fc2 = st([128, 24, 512], bf16, "fc2")
        tsink = st([1, 1], f32, "tsink")

        def xp_dma(p):
            # one issue per pair; 1 KiB contiguous per (partition, batch)
            gdma(out=xps[p],
                 in_=x_d.ap()[2 * p:2 * p + 2]
                 .rearrange("b (p l) c -> p b (l c)", p=128))

        # ---- on-chip DFT generation (index vectors; values <= 511 are
        # exact in f32, so iota straight to f32 is safe)
        nrow = st([1, 512], f32, "nrr")
        frow = st([1, FP], f32, "frr")
        for a in range(4):
            nc.gpsimd.iota(nrow[:, a * 128:(a + 1) * 128], [[4, 128]],
                           base=a, channel_multiplier=0,
                           allow_small_or_imprecise_dtypes=True)
        nc.gpsimd.iota(frow, [[1, FP]], base=0, channel_multiplier=0,
                       allow_small_or_imprecise_dtypes=True)

        # ---- DMA prologue. The gpsimd hwdge queue (q0) is by far the
        # fastest on this fleet (~230 GB/s vs ~45 for scalar's q10), so the
        # x pairs and fc1 ride it; sync takes the DFT matrices + FFT bf16
        # constants; scalar only the small f32 blob.
        for p in range(NPAIR):
            xp_dma(p)
        nc.scalar.dma_start(out=blobf, in_=blobf_d.ap())
        dma(out=blobb[:, 0:BCRIT], in_=blobb_d.ap()[:, 0:BCRIT])

        # ---- DFT trig pipeline, one chunk at a time so rfft kc=0 can
        # start the moment x lands. First ACT Sin also loads the trig
        # table (previously tsink's job).
        PI = math.pi
        MAGIC = 12582912.0          # 1.5 * 2^23: float round-to-nearest trick
        for a in range(4):
            cs = slice(a * FP, (a + 1) * FP)
            nf_ps = pt([128, FP], "zfft", 2)
            nc.tensor.matmul(nf_ps, nrow[:, a * 128:(a + 1) * 128], frow,
                             start=True, stop=True)
            # centered remainder m' = nf - 512*round(nf/512) in [-256, 256]:
            # no mod op on any engine, but f32 add of the 1.5*2^23 magic
            # constant rounds to nearest exactly (nf/512 <= 257 < 2^23).
            mmod = st([128, FP], f32, "mmod", bufs=1)
            shg = st([128, FP], f32, "shg", bufs=1)
            nc.vector.tensor_scalar(shg, nf_ps, 1.0 / 512.0, MAGIC,
                                    op0=ALU.mult, op1=ALU.add)
            nc.vector.tensor_scalar(shg, shg, -MAGIC, None, op0=ALU.add)
            nc.vector.scalar_tensor_tensor(mmod, shg, -512.0, nf_ps,
                                           op0=ALU.mult, op1=ALU.add)
            # sit = -sin(th) = sin(-m'*pi/256); crt = cos(th) = 1-2sin^2(m'*pi/512)
            sitf = st([128, FP], f32, "sitf", bufs=1)
            nc.scalar.activation(sitf, mmod, AF.Sin, scale=-PI / 256.0)
            nc.vector.tensor_scalar_mul(sitb[:, cs], sitf, 1.0)
            nc.scalar.activation(shg, mmod, AF.Sin, scale=PI / 512.0)
            nc.vector.tensor_tensor(shg, shg, shg, op=ALU.mult)
            nc.vector.tensor_scalar(crtb[:, cs], shg, -2.0, 1.0,
                                    op0=ALU.mult, op1=ALU.add)
        # zero the f=257 pad column of every chunk (garbage angles there)
        nc.vector.memset(crtb.bitcast(f32)
                         .rearrange("p (a f) -> p a f", f=FP)[:, :, 257:258], 0.0)
        nc.vector.memset(sitb.bitcast(f32)
                         .rearrange("p (a f) -> p a f", f=FP)[:, :, 257:258], 0.0)

        def fc1x_dma(j):
            gdma(out=fc1[:, 4:8, j * 768:(j + 1) * 768],
                 in_=fc1_d.ap()[512:1024, j * 768:(j + 1) * 768]
                 .rearrange("(a p) m -> p a m", p=128))

        fc1x_dma(0)
        fc1x_dma(1)

        # ---- PE warm-up: junk matmuls while the input DMAs stream, so the
        # HAM clock-gate opens (K=8/8, 2.4 GHz) before the first real matmul
        for i in range(6):
            w_ps = pt([128, 512], "pbig", 3)
            nc.tensor.matmul(w_ps, wup[:, 0:128], wup[:, 128:640],
                             start=True, stop=True)

        # core-wide activations (c_all = 512 columns = 8 batches x 64 ch)
        filt = st([128, 4, 512], bf16, "filt")      # x_filt, L-major chunks
        xbf = st([128, 4, 512], bf16, "xbf")        # x cast, L-major chunks
        ht = st([128, 4, 512], bf16, "ht")          # fan features (cos,sin,g0,g1)

        # x -> bf16 casts (z1x needs all four pairs)
        def xbf_cast(p):
            nc.vector.tensor_scalar_mul(xbf[:, :, p * 128:(p + 1) * 128],
                                        xqs[p].bitcast(f32), 1.0)

        xq_fill(0, lambda o, i: nc.vector.tensor_scalar_mul(o, i, 1.0))
        xq_fill(1, lambda o, i: nc.vector.tensor_scalar_mul(o, i, 1.0))
        xbf_cast(0)
        xbf_cast(1)

        # ---- z1x: the x-half of the MLP first layer, interleaved into the
        # FFT phase. Stash = the z1r tile itself (combined with the fan half
        # + bias later by a fused DVE op).
        z1rs = [st([128, 512], bf16, "z1r", bufs=24) for _ in range(24)]
        z1x_done = set()

        def z1x_chunk(kc):
            z1x_done.add(kc)
            zx_ps = pt([128, 512], "z2", 1)
            for a in range(4):
                nc.tensor.matmul(zx_ps, fc1[:, 4 + a, kc * 128:(kc + 1) * 128],
                                 xbf[:, a, :], start=(a == 0), stop=(a == 3))
            if kc % 2 == 0:
                nc.vector.tensor_scalar_mul(z1rs[kc], zx_ps, 1.0)
            else:
                nc.scalar.copy(z1rs[kc], zx_ps)

        # ---- attention emitters (interleaved into the MLP phase below in
        # bulk stages: long runs of same-kind matmuls keep the PE stream
        # dense so the HAM clock-gate stays open, and the ACT exp latency is
        # hidden behind interleaved z1 chunks) ----
        def attn_stage_a(p):
            # Pair-stacked operands: channel rows 0:64 = batch 2p, 64:128 =
            # 2p+1. wm is blockdiag so one K=128 matmul does both batches'
            # tT; wvo is a zero-padded per-batch stack so the output axis
            # separates batches. The per-key additive k-bias is folded into
            # the drain as a per-partition bias (tt' = tt + r).
            ftbf, ntbf = pair_data[p]
            tt_ps = pt([128, 512], "pbig", 3)
            nc.tensor.matmul(tt_ps, wm, ftbf, start=True, stop=True)
            ttbf = st([128, 512], bf16, "ttbf", bufs=2)
            nc.scalar.activation(ttbf, tt_ps, AF.Identity, bias=rcol)

            # v for both batches in one matmul per pair of L-chunks: rhs is
            # the [wvoT|0 ; 0|wvoT] block layout so output cols 0:65 are
            # batch 2p's Vcomb and 65:130 batch 2p+1's.
            v_psA = pt([128, 2, 130], "zfft", 2)
            v_psB = pt([128, 2, 130], "zfft", 2)
            for l in range(4):
                tgt = (v_psA if l < 2 else v_psB)[:, l % 2, :]
                nc.tensor.matmul(tgt, ntbf[:, l * 128:(l + 1) * 128],
                                 wvo, start=True, stop=False)
                nc.tensor.matmul(tgt, ones1, vbias, start=False, stop=True)
            vbf = st([128, 4, 130], bf16, "vbf", bufs=2)
            nc.scalar.copy(vbf[:, 0:2, :], v_psA)
            nc.scalar.copy(vbf[:, 2:4, :], v_psB)
            return (ntbf, ttbf, vbf)

        def attn_stage_b(p, state):
            # scores for both batches interleaved so the K=64 matmuls land in
            # different PE row groups (base partitions 0 and 64) and overlap.
            ntbf, ttbf, vbf = state
            ubfs = [st([128, 4, 512], bf16, "ubf", bufs=4) for _ in range(2)]
            for l in range(4):
                for j in range(2):
                    rows = slice(j * 64, (j + 1) * 64)
                    # alternate PSUM tags: zfft's slots are idle in this phase,
                    # so the 8 score matmuls aren't throttled by exp drains
                    sc_ps = pt([128, 512], "zfft" if j else "pbig",
                                2 if j else 3)
                    nc.tensor.matmul(sc_ps, ntbf[rows, l * 128:(l + 1) * 128],
                                     ttbf[rows, :], start=True, stop=True)
                    nc.scalar.activation(ubfs[j][:, l, :], sc_ps, AF.Exp)
            return (vbf, ubfs)

        def attn_stage_c(p, state):
            vbf, ubfs = state
            for j in range(2):
                b = 2 * p + j
                aot_ps = pt([65, 512], "pbig", 3)
                for l in range(4):
                    nc.tensor.matmul(aot_ps,
                                     vbf[:, l, j * 65:(j + 1) * 65],
                                     ubfs[j][:, l, :],
                                     start=(l == 0), stop=(l == 3))
                aot = st([65, 512], bf16, "aot", bufs=2)
                nc.scalar.copy(aot, aot_ps)

                aof = st([128, 4, 64], f32, "aof", bufs=2)
                for qc in range(4):
                    ao_ps = pt([128, 65], "psm", 2, bf16)
                    nc.tensor.transpose(ao_ps, aot[:, qc * 128:(qc + 1) * 128],
                                        identb[0:65, 0:65])
                    rec = st([128, 1], f32, "rec", bufs=2)
                    nc.vector.reciprocal(rec, ao_ps[:, 64:65])
                    nc.vector.tensor_scalar(aof[:, qc, :], ao_ps[:, 0:64],
                                            rec, None, op0=ALU.mult)
                dma(out=out_d.ap()[0, b].rearrange("(p l) c -> p l c", p=128),
                    in_=aof)

        def attn_group(g):
            states = [attn_stage_a(2 * g + i) for i in range(2)]
            yield
            states = [attn_stage_b(2 * g + i, s) for i, s in enumerate(states)]
            yield
            for i, s in enumerate(states):
                attn_stage_c(2 * g + i, s)

        # ================= per-pair FFT / top-k / irfft =================
        pair_data = []
        pending = []

        def tick():
            if pending:
                try:
                    next(pending[0])
                except StopIteration:
                    pending.pop(0)
                    tick()

        def fft_front(p):
            """rfft matmuls + mag^2 + top-k zap + masked bf16 spectra."""
            # fp32r (fp22-truncated) matmuls: 1 col/cycle vs fp32's 2 for
            # moving dim >= 256; plenty of mantissa for top-k selection
            zr_ps = pt([128, FP], "zfft", 2)
            zi_ps = pt([128, FP], "zfft", 2)
            for kc in range(4):
                nc.tensor.matmul(zr_ps, xqs[p][:, kc, :], crt[:, kc, :],
                                 start=(kc == 0), stop=(kc == 3))
            for kc in range(4):
                nc.tensor.matmul(zi_ps, xqs[p][:, kc, :], sit[:, kc, :],
                                 start=(kc == 0), stop=(kc == 3))

            sqr = st([128, FP], f32, "sqr", bufs=2)
            nc.scalar.square(sqr, zr_ps)
            sqi = st([128, FP], f32, "sqi", bufs=2)
            nc.scalar.square(sqi, zi_ps)
            zrbf = st([128, FP], bf16, "zrbf", bufs=2)
            nc.vector.tensor_scalar_mul(zrbf, zr_ps, 1.0)
            zibf = st([128, FP], bf16, "zibf", bufs=2)
            nc.vector.tensor_scalar_mul(zibf, zi_ps, 1.0)

            # the magnitude add runs on the (otherwise idle) gpsimd engine
            zap = st([128, FP], f32, "zap", bufs=2)
            nc.gpsimd.tensor_add(zap, sqr, sqi)
            m8 = st([128, 8], f32, "m8", bufs=2)
            for take in rounds:
                nc.vector.max(out=m8, in_=zap)
                if take < 8:
                    nc.vector.memset(m8[:, take:8], 0.0)
                nc.vector.match_replace(out=zap, in_to_replace=m8,
                                        in_values=zap, imm_value=0.0)
            # fused mask-and-apply: zrm = (zap == 0) * zrbf in one DVE op
            zrm = st([128, FP], bf16, "zrm", bufs=2)
            zim = st([128, FP], bf16, "zim", bufs=2)
            if k > 0:
                nc.vector.scalar_tensor_tensor(zrm, zap, 0.0, zrbf,
                                               op0=ALU.is_equal, op1=ALU.mult)
                nc.vector.scalar_tensor_tensor(zim, zap, 0.0, zibf,
                                               op0=ALU.is_equal, op1=ALU.mult)
            else:
                nc.vector.memset(zrm, 0.0)
                nc.vector.memset(zim, 0.0)
            return (zrm, zim)

        def fft_back(p, state):
            """transpose masked spectra, irfft, xT, normT, filt layouts."""
            zrm, zim = state
            # transpose masked spectra to F-major [257, 128]
            zmr_ps = pt([128, 384], "pbig", 3, bf16)
            zmi_ps = pt([128, 384], "pbig", 3, bf16)
            for (src, dst) in ((zrm, zmr_ps), (zim, zmi_ps)):
                nc.tensor.transpose(dst[:, 0:128], src[:, 0:128], identb)
                nc.tensor.transpose(dst[:, 128:256], src[:, 128:256], identb)
                nc.tensor.transpose(dst[0:1, 256:384], src[:, 256:257], identb)
            zmr = st([128, 384], bf16, "zmr", bufs=2)
            nc.scalar.copy(zmr[:, 0:256], zmr_ps[:, 0:256])
            nc.scalar.copy(zmr[0:1, 256:384], zmr_ps[0:1, 256:384])
            zmi = st([128, 384], bf16, "zmi", bufs=2)
            nc.scalar.copy(zmi[:, 0:256], zmi_ps[:, 0:256])
            nc.scalar.copy(zmi[0:1, 256:384], zmi_ps[0:1, 256:384])

            # irfft -> filtT [c_pair, 512] (chan-major)
            ft_ps = pt([128, 512], "pbig", 3)
            ir_ops = [(zmr[:, 0:128], art[:, 0, :]),
                      (zmr[:, 128:256], art[:, 1, :]),
                      (zmr[0:1, 256:384], art[0:1, 2, :]),
                      (zmi[:, 0:128], ait[:, 0, :]),
                      (zmi[:, 128:256], ait[:, 1, :]),
                      (zmi[0:1, 256:384], ait[0:1, 2, :])]
            for i, (lhsT, rhs) in enumerate(ir_ops):
                nc.tensor.matmul(ft_ps, lhsT, rhs, start=(i == 0),
                                 stop=(i == len(ir_ops) - 1))
            ftbf = st([128, 512], bf16, "ftbf", bufs=NPAIR)
            nc.scalar.copy(ftbf, ft_ps)

            # xT via PE transpose straight from xp (no copy in the critical
            # path); normT = xT - filtT fused on the DVE
            xt_ps = pt([128, 512], "pbig", 3)
            for l in range(4):
                nc.tensor.transpose(xt_ps[:, l * 128:(l + 1) * 128],
                                    xqs[p].bitcast(f32)[:, l, :], identf)
            ntbf = st([128, 512], bf16, "ntbf", bufs=NPAIR)
            nc.vector.tensor_sub(ntbf, xt_ps, ftbf)

            # filt L-major via PE transpose of filtT
            fl_ps = pt([128, 512], "pbig", 3, bf16)
            for l in range(4):
                nc.tensor.transpose(fl_ps[:, l * 128:(l + 1) * 128],
                                    ftbf[:, l * 128:(l + 1) * 128], identb)
            nc.scalar.copy(filt[:, :, p * 128:(p + 1) * 128],
                           fl_ps.rearrange("p (a b) -> p a b", b=128))
            pair_data.append((ftbf, ntbf))

        # FFT pipeline with z1x chunks filling the PE while the DVE runs
        # top-k; late heavy-weight DMA issues ride the gpsimd queue between
        # its per-pair magnitude adds.
        s0 = fft_front(0)
        xq_fill(2, nc.scalar.copy)
        xq_fill(3, nc.scalar.copy)
        xbf_cast(2)
        xbf_cast(3)
        s1 = fft_front(1)
        fc1x_dma(2)
        fc1x_dma(3)
        for kc in range(0, 3):
            z1x_chunk(kc)
        fft_back(0, s0)
        s2 = fft_front(2)
        gdma(out=blobb[:, BCRIT:BW], in_=blobb_d.ap()[:, BCRIT:BW])
        for kc in range(3, 6):
            z1x_chunk(kc)
        fft_back(1, s1)
        s3 = fft_front(3)
        for kc in range(6, 9):
            z1x_chunk(kc)
        gdma(out=fc1[:, 0:4, 0:1536],
             in_=fc1_d.ap()[0:512, 0:1536].rearrange("(a p) m -> p a m", p=128))
        fft_back(2, s2)
        for kc in range(9, 12):
            z1x_chunk(kc)
        gdma(out=fc1[:, 0:4, 1536:3072],
             in_=fc1_d.ap()[0:512, 1536:3072].rearrange("(a p) m -> p a m", p=128))
        fft_back(3, s3)
        for kc in range(12, 15):
            z1x_chunk(kc)
        for j in range(2):
            gdma(out=fc2[:, j * 12:(j + 1) * 12, :],
                 in_=fc2_d.ap()[j * 1536:(j + 1) * 1536, :]
                 .rearrange("(a p) m -> p a m", p=128))

        # ================= FAN (core-wide, 512 cols) =================
        pT_ps = pt([128, 512], "pbig", 3)
        for kc in range(4):
            nc.tensor.matmul(pT_ps, wpt[:, kc, :], filt[:, kc, :],
                             start=(kc == 0), stop=(kc == 3))
        # cos chunk via half-angle (ACT Sin is only valid on [-pi, pi]):
        # cos(p + bp) = 1 - 2*sin((p + bp)/2)^2
        shalf = st([128, 512], f32, "shalf")
        nc.scalar.activation(shalf, pT_ps, AF.Sin, bias=bpcs, scale=0.5)
        sh2 = st([128, 512], f32, "sh2")
        nc.scalar.square(sh2, shalf)
        nc.vector.tensor_scalar(ht[:, 0, :], sh2, -2.0, 1.0,
                                op0=ALU.mult, op1=ALU.add)
        nc.scalar.activation(ht[:, 1, :], pT_ps, AF.Sin, bias=bpss)
        for mc in range(2):
            g_ps = pt([128, 512], "pbig", 3)
            for kc in range(4):
                nc.tensor.matmul(g_ps, wgt[:, kc, mc * 128:(mc + 1) * 128],
                                 filt[:, kc, :], start=(kc == 0), stop=(kc == 3))
            nc.scalar.activation(ht[:, 2 + mc, :], g_ps, AF.Gelu,
                                 bias=bg2[:, mc:mc + 1])

        # ================= MLP (attention groups interleaved) ============
        pending.append(attn_group(0))
        pending.append(attn_group(1))
        attn_slots = {1, 4, 8, 12}

        for kc in range(24):
            if kc not in z1x_done:
                z1x_chunk(kc)
            z1_ps = pt([128, 512], "pbig", 3)
            for kk in range(4):
                nc.tensor.matmul(z1_ps, fc1[:, kk, kc * 128:(kc + 1) * 128],
                                 ht[:, kk, :], start=(kk == 0), stop=(kk == 3))
            # fused combine: z1 = (fan_psum + b1) + z1x_stash, then relu
            # (relu alternates ACT/DVE so neither engine throttles the PE)
            nc.vector.scalar_tensor_tensor(z1rs[kc], z1_ps, b1[:, kc:kc + 1],
                                           z1rs[kc], op0=ALU.add, op1=ALU.add)
            if kc % 2 == 0:
                nc.scalar.activation(z1rs[kc], z1rs[kc], AF.Relu)
            else:
                nc.vector.tensor_scalar(z1rs[kc], z1rs[kc], 0.0, None,
                                        op0=ALU.max)
            if kc in attn_slots:
                tick()

        for m in range(4):
            tick()
            # alternate PSUM banks (zfft is idle by now) so chunk m+1's
            # matmuls don't wait on chunk m's ACT drain
            z2_ps = pt([128, 512], "z2" if m % 2 == 0 else "zfft",
                       1 if m % 2 == 0 else 2)
            for kc in range(24):
                nc.tensor.matmul(z2_ps, fc2[:, kc, m * 128:(m + 1) * 128],
                                 z1rs[kc], start=(kc == 0), stop=(kc == 23))
            z2sb = st([128, 512], f32, "z2sb", bufs=2)
            if m < 3:
                nc.scalar.activation(z2sb, z2_ps, AF.Identity,
                                     bias=b2[:, m:m + 1])
                dma(out=out_d.ap()[1, :, m * 128:(m + 1) * 128, :]
                    .rearrange("b p c -> p b c"),
                    in_=z2sb.rearrange("p (b c) -> p b c", c=64))
            else:
                # last chunk: drain in halves so the final ACT+DMA overlaps
                # the preceding matmuls instead of serializing the tail
                for hh in range(2):
                    cols = slice(hh * 256, (hh + 1) * 256)
                    nc.scalar.activation(z2sb[:, cols], z2_ps[:, cols],
                                         AF.Identity, bias=b2[:, m:m + 1])
                    dma(out=out_d.ap()[1, 4 * hh:4 * (hh + 1),
                                       m * 128:(m + 1) * 128, :]
                        .rearrange("b p c -> p b c"),
                        in_=z2sb[:, cols].rearrange("p (b c) -> p b c", c=64))
        while pending:
            tick()

    nc.compile()
    return nc


def _host_inputs(inputs):
    """Host-side preprocessing -> dict of per-core-replicated input arrays
    (everything except 'x', which is sharded)."""
    f32 = np.float32
    in_proj_w = np.asarray(inputs["in_proj_w"], f32)
    in_proj_b = np.asarray(inputs["in_proj_b"], f32)
    wq, wk, wv = np.split(in_proj_w, 3, 0)
    bq, bk, bv = np.split(in_proj_b, 3, 0)
    out_w = np.asarray(inputs["out_w"], f32)
    out_b = np.asarray(inputs["out_b"], f32)

    Wm = ((wq.T / 8.0) @ wk).astype(f32)                 # [cin, cin2]
    wm2 = np.zeros((128, 128), f32)
    wm2[0:64, 0:64] = Wm
    wm2[64:128, 64:128] = Wm
    r = (wk.T @ (bq / 8.0)).astype(f32)
    wvo = out_w @ wv
    out_bp = out_b + out_w @ bv
    wvoT_ext = np.concatenate([wvo.T, np.zeros((64, 1), f32)], 1)  # [64, 65]
    wvo2 = np.zeros((128, 130), f32)
    wvo2[0:64, 0:65] = wvoT_ext
    wvo2[64:128, 65:130] = wvoT_ext
    rc = np.concatenate([r, r]).astype(f32).reshape(128, 1)
    vb = np.concatenate([out_bp, [1.0]])
    vbias_row = np.concatenate([vb, vb]).astype(BF).reshape(1, 130)

    gate = np.asarray(inputs["gate"], f32)
    gt = 1.0 / (1.0 + math.exp(-float(gate[0])))
    Wp = np.asarray(inputs["Wp"], f32)
    bp = np.asarray(inputs["bp"], f32)
    Wg = np.asarray(inputs["Wg"], f32)
    bg = np.asarray(inputs["bg"], f32)
    fc1_w = np.asarray(inputs["fc1_w"], f32)
    fc1_b = np.asarray(inputs["fc1_b"], f32)
    fc2_w = np.asarray(inputs["fc2_w"], f32)
    fc2_b = np.asarray(inputs["fc2_b"], f32)
    colscale = np.concatenate([
        np.full(128, gt), np.full(128, gt), np.full(256, 1.0 - gt), np.ones(512)
    ]).astype(f32)

    CrT, SiT, ArT, AiT = _host_dft()

    def chunked(mat, nch, width):
        """[nch*128, width] -> [128, nch*width] with chunk c at cols
        c*width:(c+1)*width (rows beyond the matrix end are zero)."""
        out = np.zeros((128, nch * width), mat.dtype)
        for c in range(nch):
            rows = mat[c * 128:(c + 1) * 128]
            out[0:rows.shape[0], c * width:(c + 1) * width] = rows
        return out

    def chunked4i(mat, width):
        """[512, width] -> [128, 4*width], chunk a = mat[a::4]: matches the
        interleaved L layout (partition p, chunk a <-> row 4p+a)."""
        out = np.zeros((128, 4 * width), mat.dtype)
        for a in range(4):
            out[:, a * width:(a + 1) * width] = mat[a::4]
        return out

    # time-axis permutation for column-indexed L (ftbf/xT col j <-> L[j])
    lperm = (np.arange(4)[:, None] + 4 * np.arange(128)[None, :]).reshape(-1)

    blobf = np.zeros((128, FW), f32)

    def putf(name, arr):
        lo, hi = FOFF[name]
        blobf[:, lo:hi] = arr

    putf("idf", np.eye(128, dtype=f32))
    putf("bpc", (bp / 2.0).reshape(128, 1))
    putf("bps", bp.reshape(128, 1))
    putf("bg", bg.reshape(2, 128).T)
    putf("b1", fc1_b.reshape(24, 128).T)
    putf("b2", fc2_b.reshape(4, 128).T)
    putf("rc", rc)

    blobb = np.zeros((128, BW), np.float32)

    def putb(name, arr):
        lo, hi = BOFF[name]
        blobb[0:arr.shape[0], lo:hi] = arr

    putb("wm", wm2)
    putb("wvo", wvo2)
    putb("idb", np.eye(128, dtype=f32))
    putb("wpt", chunked4i(Wp.T.astype(f32), 128))
    putb("wgt", chunked4i(Wg.T.astype(f32), 256))
    putb("art", chunked(ArT.astype(f32)[:, lperm], 3, SEQ))
    putb("ait", chunked(AiT.astype(f32)[:, lperm], 3, SEQ))
    putb("vb", vbias_row.astype(f32))
    putb("one", np.ones((1, 128), f32))

    fc1t = (fc1_w * colscale[None, :]).T.astype(BF).copy()
    # x-half rows follow the interleaved L layout of xbf
    fc1t[512:1024] = np.concatenate([fc1t[512:1024][a::4] for a in range(4)])
    return {
        "blobf": blobf,
        "blobb": blobb.astype(BF),
        "fc1t": fc1t,
        "fc2t": fc2_w.T.astype(BF).copy(),
    }


_RUN_KWARGS = {}   # test harness can set e.g. {"trace": True}
_LAST_RESULT = None


def kernel(**inputs):
    from concourse.bass_utils import run_bass_kernel_spmd

    k = int(np.asarray(inputs["freq_topk"]))
    if k not in _BUILD_CACHE:
        _BUILD_CACHE[k] = _build(k)
    nc = _BUILD_CACHE[k]

    const = _host_inputs(inputs)
    x = np.ascontiguousarray(np.asarray(inputs["batch_x"], np.float32))
    in_maps = []
    for c in range(NCORES):
        m = dict(const)
        m["x"] = np.ascontiguousarray(x[c * BPC:(c + 1) * BPC])
        in_maps.append(m)

    # occasional transient NRT_EXEC_UNIT_UNRECOVERABLE on this fleet; retry
    last_exc = None
    for attempt in range(3):
        try:
            res = run_bass_kernel_spmd(nc, in_maps,
                                       core_ids=list(range(NCORES)),
                                       **_RUN_KWARGS)
            outs = [np.asarray(res.results[c]["out"]) for c in range(NCORES)]
            globals()["_LAST_RESULT"] = res
            return np.concatenate(outs, axis=1).astype(np.float32)
        except Exception as e:  # noqa: BLE001
            last_exc = e
            import time
            time.sleep(2.0 * (attempt + 1))
    raise last_exc


if __name__ == "__main__":
    d = np.load("/tmp/ref_inputs.npz")
    inputs = {kk: d[kk] for kk in d.files}
    out = kernel(**inputs)
    ref = np.load("/tmp/ref_out.npy")
    rel = np.linalg.norm(out - ref) / np.linalg.norm(ref)
    print("rel err:", rel)


# revision 45
# speedup vs baseline: 1.0648x; 1.0648x over previous
"""Trainium2 Bass kernel for nn_AEFIN (FFT top-k masking + attention + FAN/MLP).

Data-parallel over batch: 64 batches sharded 8-per-core across 8 NeuronCores.
Inside each core (all shapes hardcoded for BS=64, L=512, E=64, pred=512):

  per pair of batches (c_pair = 2*64 = 128 channels packed on partitions):
    rfft as f32 matmul (exact enough for top-k selection)   [PE, f32]
    mag^2 -> 8-at-a-time max + match_replace top-k zap      [DVE]
    mask = (zap==0); masked spectra in bf16                 [DVE]
    irfft as bf16 matmul -> x_filt (both layouts), norm     [PE]
  per batch: single-head attention with host-folded weights [PE/ACT, bf16]
  core-wide: FAN (sin/cos/gelu) + 2-layer MLP               [PE/ACT, bf16]

Schedule: the x-half of the MLP's first layer (z1x = fc1[:,512:] @ x) depends
only on the input and fc1 weights, so its 96 matmuls are interleaved into the
FFT/top-k phase (which otherwise idles the PE while the DVE runs top-k rounds
and lets the HAM clock gate re-throttle to 1.2 GHz).  z1x chunks stash to the
z1r tiles in bf16; the fan-half accumulates later and a fused DVE
scalar_tensor_tensor adds stash + bias before the relu.

Host-side folds: attention score matrix Wm = (wq/8)^T wk, k-bias folded as a
per-partition bias on the ttbf drain (tt' = tt + r adds r.n_key to every
score column == the additive per-key bias), v/out_proj merged (wvo = out_w @
wv, bias folded through attn row-sum), FAN gate sigmoid folded into fc1
columns, all weight transposes done on host.
"""

import math
import os
import sys

for _p in ("/opt/trn_rl_repo",):
    if _p not in sys.path and os.path.isdir(_p):
        sys.path.append(_p)

import numpy as np
import ml_dtypes

BF = ml_dtypes.bfloat16
SEQ, PRED, E, BS = 512, 512, 64, 64
F = SEQ // 2 + 1  # 257
FP = F + 1        # 258: fp32r matmuls need an even moving size
NCORES = 8
BPC = BS // NCORES       # batches per core = 8
NPAIR = BPC // 2         # 4


def _host_dft():
    n = np.arange(SEQ, dtype=np.float64)
    f = np.arange(F, dtype=np.float64)
    ang = 2.0 * np.pi * np.outer(n, f) / SEQ            # [512, 257]
    CrT = np.cos(ang).astype(np.float32)
    SiT = (-np.sin(ang)).astype(np.float32)
    w = np.full(F, 2.0 / SEQ)
    w[0] = 1.0 / SEQ
    w[-1] = 1.0 / SEQ
    angT = ang.T                                        # [257, 512]
    ArT = (w[:, None] * np.cos(angT)).astype(BF)
    AiT = (-(w[:, None]) * np.sin(angT)).astype(BF)
    return CrT, SiT, ArT, AiT


def _mk_layout(entries):
    off, c = {}, 0
    for name, w in entries:
        off[name] = (c, c + w)
        c += w
    return off, c


# f32 constant blob columns: f32 identity + biases + attn k-bias column
FOFF, FW = _mk_layout([
    ("idf", 128), ("bpc", 1), ("bps", 1), ("bg", 2), ("b1", 24), ("b2", 4),
    ("rc", 1),
])
# bf16 constant blob columns. First section (idb/art/ait) is needed by the
# FFT phase and is DMA'd first; the rest arrives later.
BOFF, BW = _mk_layout([
    ("idb", 128), ("art", 3 * SEQ), ("ait", 3 * SEQ),
    ("wm", 128), ("wvo", 130), ("wpt", 4 * 128), ("wgt", 4 * 256),
    ("vb", 130), ("one", 128),
])
BCRIT = BOFF["ait"][1]   # end of the FFT-critical section

_BUILD_CACHE = {}


def _build(k):
    """Build the SPMD Bass graph (identical on all cores). Returns nc."""
    import concourse.mybir as mybir
    import concourse.tile as tile
    from concourse import bacc
    from contextlib import ExitStack

    f32 = mybir.dt.float32
    f32r = mybir.dt.float32r
    bf16 = mybir.dt.bfloat16
    AF = mybir.ActivationFunctionType
    ALU = mybir.AluOpType

    nc = bacc.Bacc("TRN2", target_bir_lowering=False)

    # ---- DRAM parameters (per-core) ----
    P = {}

    def dparam(name, shape, dt):
        P[name] = nc.declare_dram_parameter(name, list(shape), dt, isOutput=False)
        return P[name]

    x_d = dparam("x", [BPC, SEQ, E], f32r)
    blobf_d = dparam("blobf", [128, FW], f32)
    blobb_d = dparam("blobb", [128, BW], bf16)
    fc1_d = dparam("fc1t", [1024, 3072], bf16)
    fc2_d = dparam("fc2t", [3072, 512], bf16)
    out_d = nc.declare_dram_parameter("out", [2, BPC, SEQ, E], f32, isOutput=True)

    rounds = []
    rem = k
    while rem > 0:
        rounds.append(min(8, rem))
        rem -= 8

    with tile.TileContext(nc) as tc, ExitStack() as ctx:
        sb = ctx.enter_context(tc.tile_pool(name="sb", bufs=1))
        ps = ctx.enter_context(tc.tile_pool(name="ps", bufs=1, space="PSUM"))

        def st(shape, dt, tag, bufs=1):
            return sb.tile(shape, dt, tag=tag, bufs=bufs, name=tag)

        def pt(shape, tag, bufs, dt=None):
            return ps.tile(shape, dt or mybir.dt.float32, tag=tag, bufs=bufs,
                           name=tag)

        dma = nc.sync.dma_start
        gdma = nc.gpsimd.dma_start

        # ---- SBUF tiles for constants ----
        # crtb/sitb (the forward DFT) are GENERATED on-chip during the DMA
        # head: nf = n x f (rank-1 f32r matmul, exact), m = nf mod 512,
        # sit = -sin(2pi n f/512) = sin(m*pi/256 - pi),
        # crt =  cos(2pi n f/512) = 2*sin(m*pi/512 - pi/2)^2 - 1.
        crtb = st([128, 4 * FP], f32r, "crtb")
        sitb = st([128, 4 * FP], f32r, "sitb")
        blobf = st([128, FW], f32, "blobf")
        blobb = st([128, BW], bf16, "blobb")

        def fview(lo, hi):
            return blobf[:, lo:hi]

        crt = crtb.rearrange("p (a f) -> p a f", f=FP)
        sit = sitb.rearrange("p (a f) -> p a f", f=FP)
        identf = fview(*FOFF["idf"])
        bpcs = fview(*FOFF["bpc"])
        bpss = fview(*FOFF["bps"])
        bg2 = fview(*FOFF["bg"])
        b1 = fview(*FOFF["b1"])
        b2 = fview(*FOFF["b2"])
        rcol = fview(*FOFF["rc"])

        def bview(lo, hi):
            return blobb[:, lo:hi]

        wm = bview(*BOFF["wm"])
        wvo = bview(*BOFF["wvo"])
        identb = bview(*BOFF["idb"])
        wpt = bview(*BOFF["wpt"]).rearrange("p (a m) -> p a m", m=128)
        wgt = bview(*BOFF["wgt"]).rearrange("p (a m) -> p a m", m=256)
        art = bview(*BOFF["art"]).rearrange("p (a m) -> p a m", m=SEQ)
        ait = bview(*BOFF["ait"]).rearrange("p (a m) -> p a m", m=SEQ)
        vbias = blobb[0:1, BOFF["vb"][0]:BOFF["vb"][1]]
        ones1 = blobb[0:1, BOFF["one"][0]:BOFF["one"][1]]

        wup = st([128, 512], bf16, "wup")
        nc.vector.memset(wup, 1.0)

        # per-pair input x: [part p, batch j, (l c)] with partition p holding
        # time rows 4p..4p+3 contiguously -> 1 KiB DMA descriptors. The
        # interleaved-L semantics propagate to all L-indexed constants
        # (host-permuted, chunk a = rows a::4).
        xps = [st([128, 2, 256], f32r, "xp", bufs=NPAIR) for _ in range(NPAIR)]
        # on-chip reshuffle to [p, l, (j c)]: the 2D-contiguous DMA layout
        # keeps descriptors at 1 KiB; the copies stay f32r end-to-end (the
        # BIR verifier requires f32r-rounded producers for f32r matmuls)
        xqs = [st([128, 4, 128], f32r, "xq", bufs=NPAIR) for _ in range(NPAIR)]

        def xq_fill(p, eng):
            for l in range(4):
                eng(xqs[p][:, l, :].rearrange("p (j c) -> p j c", j=2),
                    xps[p][:, :, l * 64:(l + 1) * 64])
        fc1 = st([128, 8, 3072], bf16, "fc1")
        fc2 = st([128, 24, 512], bf16, "fc2")
        tsink = st([1, 1], f32, "tsink")

        def xp_dma(p):
            # one issue per pair; 1 KiB contiguous per (partition, batch)
            gdma(out=xps[p],
                 in_=x_d.ap()[2 * p:2 * p + 2]
                 .rearrange("b (p l) c -> p b (l c)", p=128))

        # ---- on-chip DFT generation (index vectors; values <= 511 are
        # exact in f32, so iota straight to f32 is safe). One 2D-pattern
        # iota builds all four n-chunks (value 4p+a at position a*128+p).
        nrowf = st([1, 512], f32, "nrf")
        frowf = st([1, FP], f32, "frf")
        nc.gpsimd.iota(nrowf, [[1, 4], [4, 128]], base=0,
                       channel_multiplier=0,
                       allow_small_or_imprecise_dtypes=True)
        nc.gpsimd.iota(frowf, [[1, FP]], base=0, channel_multiplier=0,
                       allow_small_or_imprecise_dtypes=True)
        nrow = st([1, 512], f32r, "nrr")
        frow = st([1, FP], f32r, "frr")
        nc.vector.tensor_scalar_mul(nrow, nrowf, 1.0)
        nc.vector.tensor_scalar_mul(frow, frowf, 1.0)

        # ---- DMA prologue. The gpsimd hwdge queue (q0) is by far the
        # fastest on this fleet (~230 GB/s vs ~45 for scalar's q10), so the
        # x pairs and fc1 ride it; sync takes the DFT matrices + FFT bf16
        # constants; scalar only the small f32 blob.
        for p in range(NPAIR):
            xp_dma(p)
        nc.scalar.dma_start(out=blobf, in_=blobf_d.ap())
        dma(out=blobb[:, 0:BCRIT], in_=blobb_d.ap()[:, 0:BCRIT])

        # ---- DFT trig pipeline, one chunk at a time so rfft kc=0 can
        # start the moment x lands. First ACT Sin also loads the trig
        # table (previously tsink's job).
        PI = math.pi
        MAGIC = 12582912.0          # 1.5 * 2^23: float round-to-nearest trick
        for a in range(4):
            cs = slice(a * FP, (a + 1) * FP)
            nf_ps = pt([128, FP], "zfft", 2)
            nc.tensor.matmul(nf_ps, nrow[:, a * 128:(a + 1) * 128], frow,
                             start=True, stop=True)
            # centered remainder m' = nf - 512*round(nf/512) in [-256, 256]:
            # no mod op on any engine, but f32 add of the 1.5*2^23 magic
            # constant rounds to nearest exactly (nf/512 <= 257 < 2^23).
            mmod = st([128, FP], f32, "mmod", bufs=2)
            shg = st([128, FP], f32, "shg", bufs=2)
            nc.vector.tensor_scalar(shg, nf_ps, 1.0 / 512.0, MAGIC,
                                    op0=ALU.mult, op1=ALU.add)
            nc.vector.tensor_scalar(shg, shg, -MAGIC, None, op0=ALU.add)
            nc.vector.scalar_tensor_tensor(mmod, shg, -512.0, nf_ps,
                                           op0=ALU.mult, op1=ALU.add)
            # sit = -sin(th) = sin(-m'*pi/256); crt = cos(th) = 1-2sin^2(m'*pi/512)
            nc.scalar.activation(sitb[:, cs], mmod, AF.Sin, scale=-PI / 256.0)
            nc.scalar.activation(shg, mmod, AF.Sin, scale=PI / 512.0)
            nc.vector.tensor_tensor(shg, shg, shg, op=ALU.mult)
            nc.vector.tensor_scalar(crtb[:, cs], shg, -2.0, 1.0,
                                    op0=ALU.mult, op1=ALU.add)
        # zero the f=257 pad column of every chunk (garbage angles there)
        nc.vector.memset(crtb.bitcast(f32)
                         .rearrange("p (a f) -> p a f", f=FP)[:, :, 257:258], 0.0)
        nc.vector.memset(sitb.bitcast(f32)
                         .rearrange("p (a f) -> p a f", f=FP)[:, :, 257:258], 0.0)

        def fc1x_dma(j):
            gdma(out=fc1[:, 4:8, j * 768:(j + 1) * 768],
                 in_=fc1_d.ap()[512:1024, j * 768:(j + 1) * 768]
                 .rearrange("(a p) m -> p a m", p=128))

        fc1x_dma(0)
        fc1x_dma(1)

        # ---- PE warm-up: junk matmuls while the input DMAs stream, so the
        # HAM clock-gate opens (K=8/8, 2.4 GHz) before the first real matmul
        for i in range(6):
            w_ps = pt([128, 512], "pbig", 3)
            nc.tensor.matmul(w_ps, wup[:, 0:128], wup[:, 0:512],
                             start=True, stop=True)

        # core-wide activations (c_all = 512 columns = 8 batches x 64 ch)
        filt = st([128, 4, 512], bf16, "filt")      # x_filt, L-major chunks
        xbf = st([128, 4, 512], bf16, "xbf")        # x cast, L-major chunks
        ht = st([128, 4, 512], bf16, "ht")          # fan features (cos,sin,g0,g1)

        # x -> bf16 casts (z1x needs all four pairs)
        def xbf_cast(p):
            nc.vector.tensor_scalar_mul(xbf[:, :, p * 128:(p + 1) * 128],
                                        xqs[p].bitcast(f32), 1.0)

        xq_fill(0, lambda o, i: nc.vector.tensor_scalar_mul(o, i, 1.0))
        xq_fill(1, lambda o, i: nc.vector.tensor_scalar_mul(o, i, 1.0))
        xbf_cast(0)
        xbf_cast(1)

        # ---- z1x: the x-half of the MLP first layer, interleaved into the
        # FFT phase. Stash = the z1r tile itself (combined with the fan half
        # + bias later by a fused DVE op).
        z1rs = [st([128, 512], bf16, "z1r", bufs=24) for _ in range(24)]
        z1x_done = set()

        def z1x_chunk(kc):
            z1x_done.add(kc)
            zx_ps = pt([128, 512], "z2", 1)
            for a in range(4):
                nc.tensor.matmul(zx_ps, fc1[:, 4 + a, kc * 128:(kc + 1) * 128],
                                 xbf[:, a, :], start=(a == 0), stop=(a == 3))
            if kc % 2 == 0:
                nc.vector.tensor_scalar_mul(z1rs[kc], zx_ps, 1.0)
            else:
                nc.scalar.copy(z1rs[kc], zx_ps)

        # ---- attention emitters (interleaved into the MLP phase below in
        # bulk stages: long runs of same-kind matmuls keep the PE stream
        # dense so the HAM clock-gate stays open, and the ACT exp latency is
        # hidden behind interleaved z1 chunks) ----
        def attn_stage_a(p):
            # Pair-stacked operands: channel rows 0:64 = batch 2p, 64:128 =
            # 2p+1. wm is blockdiag so one K=128 matmul does both batches'
            # tT; wvo is a zero-padded per-batch stack so the output axis
            # separates batches. The per-key additive k-bias is folded into
            # the drain as a per-partition bias (tt' = tt + r).
            ftbf, ntbf = pair_data[p]
            tt_ps = pt([128, 512], "pbig", 3)
            nc.tensor.matmul(tt_ps, wm, ftbf, start=True, stop=True)
            ttbf = st([128, 512], bf16, "ttbf", bufs=2)
            nc.scalar.activation(ttbf, tt_ps, AF.Identity, bias=rcol)

            # v for both batches in one matmul per pair of L-chunks: rhs is
            # the [wvoT|0 ; 0|wvoT] block layout so output cols 0:65 are
            # batch 2p's Vcomb and 65:130 batch 2p+1's.
            v_psA = pt([128, 2, 130], "zfft", 2)
            v_psB = pt([128, 2, 130], "zfft", 2)
            for l in range(4):
                tgt = (v_psA if l < 2 else v_psB)[:, l % 2, :]
                nc.tensor.matmul(tgt, ntbf[:, l * 128:(l + 1) * 128],
                                 wvo, start=True, stop=False)
                nc.tensor.matmul(tgt, ones1, vbias, start=False, stop=True)
            vbf = st([128, 4, 130], bf16, "vbf", bufs=2)
            nc.scalar.copy(vbf[:, 0:2, :], v_psA)
            nc.scalar.copy(vbf[:, 2:4, :], v_psB)
            return (ntbf, ttbf, vbf)

        def attn_stage_b(p, state):
            # scores for both batches interleaved so the K=64 matmuls land in
            # different PE row groups (base partitions 0 and 64) and overlap.
            ntbf, ttbf, vbf = state
            ubfs = [st([128, 4, 512], bf16, "ubf", bufs=4) for _ in range(2)]
            for l in range(4):
                for j in range(2):
                    rows = slice(j * 64, (j + 1) * 64)
                    # alternate PSUM tags: zfft's slots are idle in this phase,
                    # so the 8 score matmuls aren't throttled by exp drains
                    sc_ps = pt([128, 512], "zfft" if j else "pbig",
                                2 if j else 3)
                    nc.tensor.matmul(sc_ps, ntbf[rows, l * 128:(l + 1) * 128],
                                     ttbf[rows, :], start=True, stop=True)
                    nc.scalar.activation(ubfs[j][:, l, :], sc_ps, AF.Exp)
            return (vbf, ubfs)

        def attn_stage_c(p, state):
            vbf, ubfs = state
            for j in range(2):
                b = 2 * p + j
                aot_ps = pt([65, 512], "pbig", 3)
                for l in range(4):
                    nc.tensor.matmul(aot_ps,
                                     vbf[:, l, j * 65:(j + 1) * 65],
                                     ubfs[j][:, l, :],
                                     start=(l == 0), stop=(l == 3))
                aot = st([65, 512], bf16, "aot", bufs=2)
                nc.scalar.copy(aot, aot_ps)

                aof = st([128, 4, 64], f32, "aof", bufs=2)
                for qc in range(4):
                    ao_ps = pt([128, 65], "psm", 2, bf16)
                    nc.tensor.transpose(ao_ps, aot[:, qc * 128:(qc + 1) * 128],
                                        identb[0:65, 0:65])
                    rec = st([128, 1], f32, "rec", bufs=2)
                    nc.vector.reciprocal(rec, ao_ps[:, 64:65])
                    nc.vector.tensor_scalar(aof[:, qc, :], ao_ps[:, 0:64],
                                            rec, None, op0=ALU.mult)
                dma(out=out_d.ap()[0, b].rearrange("(p l) c -> p l c", p=128),
                    in_=aof)

        def attn_group(g):
            states = [attn_stage_a(2 * g + i) for i in range(2)]
            yield
            states = [attn_stage_b(2 * g + i, s) for i, s in enumerate(states)]
            yield
            for i, s in enumerate(states):
                attn_stage_c(2 * g + i, s)

        # ================= per-pair FFT / top-k / irfft =================
        pair_data = []
        pending = []

        def tick():
            if pending:
                try:
                    next(pending[0])
                except StopIteration:
                    pending.pop(0)
                    tick()

        def fft_front(p):
            """rfft matmuls + mag^2 + top-k zap + masked bf16 spectra."""
            # fp32r (fp22-truncated) matmuls: 1 col/cycle vs fp32's 2 for
            # moving dim >= 256; plenty of mantissa for top-k selection
            zr_ps = pt([128, FP], "zfft", 2)
            zi_ps = pt([128, FP], "zfft", 2)
            for kc in range(4):
                nc.tensor.matmul(zr_ps, xqs[p][:, kc, :], crt[:, kc, :],
                                 start=(kc == 0), stop=(kc == 3))
            for kc in range(4):
                nc.tensor.matmul(zi_ps, xqs[p][:, kc, :], sit[:, kc, :],
                                 start=(kc == 0), stop=(kc == 3))

            sqr = st([128, FP], f32, "sqr", bufs=2)
            nc.scalar.square(sqr, zr_ps)
            sqi = st([128, FP], f32, "sqi", bufs=2)
            nc.scalar.square(sqi, zi_ps)
            zrbf = st([128, FP], bf16, "zrbf", bufs=2)
            nc.vector.tensor_scalar_mul(zrbf, zr_ps, 1.0)
            zibf = st([128, FP], bf16, "zibf", bufs=2)
            nc.vector.tensor_scalar_mul(zibf, zi_ps, 1.0)

            # the magnitude add runs on the (otherwise idle) gpsimd engine
            zap = st([128, FP], f32, "zap", bufs=2)
            nc.gpsimd.tensor_add(zap, sqr, sqi)
            m8 = st([128, 8], f32, "m8", bufs=2)
            for take in rounds:
                nc.vector.max(out=m8, in_=zap)
                if take < 8:
                    nc.vector.memset(m8[:, take:8], 0.0)
                nc.vector.match_replace(out=zap, in_to_replace=m8,
                                        in_values=zap, imm_value=0.0)
            # fused mask-and-apply: zrm = (zap == 0) * zrbf in one DVE op
            zrm = st([128, FP], bf16, "zrm", bufs=2)
            zim = st([128, FP], bf16, "zim", bufs=2)
            if k > 0:
                nc.vector.scalar_tensor_tensor(zrm, zap, 0.0, zrbf,
                                               op0=ALU.is_equal, op1=ALU.mult)
                nc.vector.scalar_tensor_tensor(zim, zap, 0.0, zibf,
                                               op0=ALU.is_equal, op1=ALU.mult)
            else:
                nc.vector.memset(zrm, 0.0)
                nc.vector.memset(zim, 0.0)
            return (zrm, zim)

        def fft_back(p, state):
            """transpose masked spectra, irfft, xT, normT, filt layouts."""
            zrm, zim = state
            # transpose masked spectra to F-major [257, 128]
            zmr_ps = pt([128, 384], "pbig", 3, bf16)
            zmi_ps = pt([128, 384], "pbig", 3, bf16)
            for (src, dst) in ((zrm, zmr_ps), (zim, zmi_ps)):
                nc.tensor.transpose(dst[:, 0:128], src[:, 0:128], identb)
                nc.tensor.transpose(dst[:, 128:256], src[:, 128:256], identb)
                nc.tensor.transpose(dst[0:1, 256:384], src[:, 256:257], identb)
            zmr = st([128, 384], bf16, "zmr", bufs=2)
            nc.scalar.copy(zmr[:, 0:256], zmr_ps[:, 0:256])
            nc.scalar.copy(zmr[0:1, 256:384], zmr_ps[0:1, 256:384])
            zmi = st([128, 384], bf16, "zmi", bufs=2)
            nc.scalar.copy(zmi[:, 0:256], zmi_ps[:, 0:256])
            nc.scalar.copy(zmi[0:1, 256:384], zmi_ps[0:1, 256:384])

            # irfft -> filtT [c_pair, 512] (chan-major)
            ft_ps = pt([128, 512], "pbig", 3)
            ir_ops = [(zmr[:, 0:128], art[:, 0, :]),
                      (zmr[:, 128:256], art[:, 1, :]),
                      (zmr[0:1, 256:384], art[0:1, 2, :]),
                      (zmi[:, 0:128], ait[:, 0, :]),
                      (zmi[:, 128:256], ait[:, 1, :]),
                      (zmi[0:1, 256:384], ait[0:1, 2, :])]
            for i, (lhsT, rhs) in enumerate(ir_ops):
                nc.tensor.matmul(ft_ps, lhsT, rhs, start=(i == 0),
                                 stop=(i == len(ir_ops) - 1))
            ftbf = st([128, 512], bf16, "ftbf", bufs=NPAIR)
            nc.scalar.copy(ftbf, ft_ps)

            # xT via PE transpose straight from xp (no copy in the critical
            # path); normT = xT - filtT fused on the DVE
            xt_ps = pt([128, 512], "pbig", 3)
            for l in range(4):
                nc.tensor.transpose(xt_ps[:, l * 128:(l + 1) * 128],
                                    xqs[p].bitcast(f32)[:, l, :], identf)
            ntbf = st([128, 512], bf16, "ntbf", bufs=NPAIR)
            nc.vector.tensor_sub(ntbf, xt_ps, ftbf)

            # filt L-major via PE transpose of filtT
            fl_ps = pt([128, 512], "pbig", 3, bf16)
            for l in range(4):
                nc.tensor.transpose(fl_ps[:, l * 128:(l + 1) * 128],
                                    ftbf[:, l * 128:(l + 1) * 128], identb)
            nc.scalar.copy(filt[:, :, p * 128:(p + 1) * 128],
                           fl_ps.rearrange("p (a b) -> p a b", b=128))
            pair_data.append((ftbf, ntbf))

        # FFT pipeline with z1x chunks filling the PE while the DVE runs
        # top-k; late heavy-weight DMA issues ride the gpsimd queue between
        # its per-pair magnitude adds.
        s0 = fft_front(0)
        xq_fill(2, nc.scalar.copy)
        xq_fill(3, nc.scalar.copy)
        xbf_cast(2)
        xbf_cast(3)
        s1 = fft_front(1)
        fc1x_dma(2)
        fc1x_dma(3)
        for kc in range(0, 3):
            z1x_chunk(kc)
        fft_back(0, s0)
        s2 = fft_front(2)
        gdma(out=blobb[:, BCRIT:BW], in_=blobb_d.ap()[:, BCRIT:BW])
        for kc in range(3, 6):
            z1x_chunk(kc)
        fft_back(1, s1)
        s3 = fft_front(3)
        for kc in range(6, 9):
            z1x_chunk(kc)
        gdma(out=fc1[:, 0:4, 0:1536],
             in_=fc1_d.ap()[0:512, 0:1536].rearrange("(a p) m -> p a m", p=128))
        fft_back(2, s2)
        for kc in range(9, 12):
            z1x_chunk(kc)
        gdma(out=fc1[:, 0:4, 1536:3072],
             in_=fc1_d.ap()[0:512, 1536:3072].rearrange("(a p) m -> p a m", p=128))
        fft_back(3, s3)
        for kc in range(12, 15):
            z1x_chunk(kc)
        for j in range(2):
            gdma(out=fc2[:, j * 12:(j + 1) * 12, :],
                 in_=fc2_d.ap()[j * 1536:(j + 1) * 1536, :]
                 .rearrange("(a p) m -> p a m", p=128))

        # ================= FAN (core-wide, 512 cols) =================
        pT_ps = pt([128, 512], "pbig", 3)
        for kc in range(4):
            nc.tensor.matmul(pT_ps, wpt[:, kc, :], filt[:, kc, :],
                             start=(kc == 0), stop=(kc == 3))
        # cos chunk via half-angle (ACT Sin is only valid on [-pi, pi]):
        # cos(p + bp) = 1 - 2*sin((p + bp)/2)^2
        shalf = st([128, 512], f32, "shalf")
        nc.scalar.activation(shalf, pT_ps, AF.Sin, bias=bpcs, scale=0.5)
        sh2 = st([128, 512], f32, "sh2")
        nc.scalar.square(sh2, shalf)
        nc.vector.tensor_scalar(ht[:, 0, :], sh2, -2.0, 1.0,
                                op0=ALU.mult, op1=ALU.add)
        nc.scalar.activation(ht[:, 1, :], pT_ps, AF.Sin, bias=bpss)
        for mc in range(2):
            g_ps = pt([128, 512], "pbig", 3)
            for kc in range(4):
                nc.tensor.matmul(g_ps, wgt[:, kc, mc * 128:(mc + 1) * 128],
                                 filt[:, kc, :], start=(kc == 0), stop=(kc == 3))
            nc.scalar.activation(ht[:, 2 + mc, :], g_ps, AF.Gelu,
                                 bias=bg2[:, mc:mc + 1])

        # ================= MLP (attention groups interleaved) ============
        pending.append(attn_group(0))
        pending.append(attn_group(1))
        attn_slots = {1, 4, 8, 12}

        for kc in range(24):
            if kc not in z1x_done:
                z1x_chunk(kc)
            z1_ps = pt([128, 512], "pbig", 3)
            for kk in range(4):
                nc.tensor.matmul(z1_ps, fc1[:, kk, kc * 128:(kc + 1) * 128],
                                 ht[:, kk, :], start=(kk == 0), stop=(kk == 3))
            # fused combine: z1 = (fan_psum + b1) + z1x_stash, then relu
            # (relu alternates ACT/DVE so neither engine throttles the PE)
            nc.vector.scalar_tensor_tensor(z1rs[kc], z1_ps, b1[:, kc:kc + 1],
                                           z1rs[kc], op0=ALU.add, op1=ALU.add)
            if kc % 2 == 0:
                nc.scalar.activation(z1rs[kc], z1rs[kc], AF.Relu)
            else:
                nc.vector.tensor_scalar(z1rs[kc], z1rs[kc], 0.0, None,
                                        op0=ALU.max)
            if kc in attn_slots:
                tick()

        for m in range(4):
            tick()
            # alternate PSUM banks (zfft is idle by now) so chunk m+1's
            # matmuls don't wait on chunk m's ACT drain
            z2_ps = pt([128, 512], "z2" if m % 2 == 0 else "zfft",
                       1 if m % 2 == 0 else 2)
            for kc in range(24):
                nc.tensor.matmul(z2_ps, fc2[:, kc, m * 128:(m + 1) * 128],
                                 z1rs[kc], start=(kc == 0), stop=(kc == 23))
            z2sb = st([128, 512], f32, "z2sb", bufs=2)
            if m < 3:
                nc.scalar.activation(z2sb, z2_ps, AF.Identity,
                                     bias=b2[:, m:m + 1])
                dma(out=out_d.ap()[1, :, m * 128:(m + 1) * 128, :]
                    .rearrange("b p c -> p b c"),
                    in_=z2sb.rearrange("p (b c) -> p b c", c=64))
            else:
                # last chunk: drain in halves so the final ACT+DMA overlaps
                # the preceding matmuls instead of serializing the tail
                for hh in range(2):
                    cols = slice(hh * 256, (hh + 1) * 256)
                    nc.scalar.activation(z2sb[:, cols], z2_ps[:, cols],
                                         AF.Identity, bias=b2[:, m:m + 1])
                    dma(out=out_d.ap()[1, 4 * hh:4 * (hh + 1),
                                       m * 128:(m + 1) * 128, :]
                        .rearrange("b p c -> p b c"),
                        in_=z2sb[:, cols].rearrange("p (b c) -> p b c", c=64))
        while pending:
            tick()

    nc.compile()
    return nc


def _host_inputs(inputs):
    """Host-side preprocessing -> dict of per-core-replicated input arrays
    (everything except 'x', which is sharded)."""
    f32 = np.float32
    in_proj_w = np.asarray(inputs["in_proj_w"], f32)
    in_proj_b = np.asarray(inputs["in_proj_b"], f32)
    wq, wk, wv = np.split(in_proj_w, 3, 0)
    bq, bk, bv = np.split(in_proj_b, 3, 0)
    out_w = np.asarray(inputs["out_w"], f32)
    out_b = np.asarray(inputs["out_b"], f32)

    Wm = ((wq.T / 8.0) @ wk).astype(f32)                 # [cin, cin2]
    wm2 = np.zeros((128, 128), f32)
    wm2[0:64, 0:64] = Wm
    wm2[64:128, 64:128] = Wm
    r = (wk.T @ (bq / 8.0)).astype(f32)
    wvo = out_w @ wv
    out_bp = out_b + out_w @ bv
    wvoT_ext = np.concatenate([wvo.T, np.zeros((64, 1), f32)], 1)  # [64, 65]
    wvo2 = np.zeros((128, 130), f32)
    wvo2[0:64, 0:65] = wvoT_ext
    wvo2[64:128, 65:130] = wvoT_ext
    rc = np.concatenate([r, r]).astype(f32).reshape(128, 1)
    vb = np.concatenate([out_bp, [1.0]])
    vbias_row = np.concatenate([vb, vb]).astype(BF).reshape(1, 130)

    gate = np.asarray(inputs["gate"], f32)
    gt = 1.0 / (1.0 + math.exp(-float(gate[0])))
    Wp = np.asarray(inputs["Wp"], f32)
    bp = np.asarray(inputs["bp"], f32)
    Wg = np.asarray(inputs["Wg"], f32)
    bg = np.asarray(inputs["bg"], f32)
    fc1_w = np.asarray(inputs["fc1_w"], f32)
    fc1_b = np.asarray(inputs["fc1_b"], f32)
    fc2_w = np.asarray(inputs["fc2_w"], f32)
    fc2_b = np.asarray(inputs["fc2_b"], f32)
    colscale = np.concatenate([
        np.full(128, gt), np.full(128, gt), np.full(256, 1.0 - gt), np.ones(512)
    ]).astype(f32)

    CrT, SiT, ArT, AiT = _host_dft()

    def chunked(mat, nch, width):
        """[nch*128, width] -> [128, nch*width] with chunk c at cols
        c*width:(c+1)*width (rows beyond the matrix end are zero)."""
        out = np.zeros((128, nch * width), mat.dtype)
        for c in range(nch):
            rows = mat[c * 128:(c + 1) * 128]
            out[0:rows.shape[0], c * width:(c + 1) * width] = rows
        return out

    def chunked4i(mat, width):
        """[512, width] -> [128, 4*width], chunk a = mat[a::4]: matches the
        interleaved L layout (partition p, chunk a <-> row 4p+a)."""
        out = np.zeros((128, 4 * width), mat.dtype)
        for a in range(4):
            out[:, a * width:(a + 1) * width] = mat[a::4]
        return out

    # time-axis permutation for column-indexed L (ftbf/xT col j <-> L[j])
    lperm = (np.arange(4)[:, None] + 4 * np.arange(128)[None, :]).reshape(-1)

    blobf = np.zeros((128, FW), f32)

    def putf(name, arr):
        lo, hi = FOFF[name]
        blobf[:, lo:hi] = arr

    putf("idf", np.eye(128, dtype=f32))
    putf("bpc", (bp / 2.0).reshape(128, 1))
    putf("bps", bp.reshape(128, 1))
    putf("bg", bg.reshape(2, 128).T)
    putf("b1", fc1_b.reshape(24, 128).T)
    putf("b2", fc2_b.reshape(4, 128).T)
    putf("rc", rc)

    blobb = np.zeros((128, BW), np.float32)

    def putb(name, arr):
        lo, hi = BOFF[name]
        blobb[0:arr.shape[0], lo:hi] = arr

    putb("wm", wm2)
    putb("wvo", wvo2)
    putb("idb", np.eye(128, dtype=f32))
    putb("wpt", chunked4i(Wp.T.astype(f32), 128))
    putb("wgt", chunked4i(Wg.T.astype(f32), 256))
    putb("art", chunked(ArT.astype(f32)[:, lperm], 3, SEQ))
    putb("ait", chunked(AiT.astype(f32)[:, lperm], 3, SEQ))
    putb("vb", vbias_row.astype(f32))
    putb("one", np.ones((1, 128), f32))

    fc1t = (fc1_w * colscale[None, :]).T.astype(BF).copy()
    # x-half rows follow the interleaved L layout of xbf
    fc1t[512:1024] = np.concatenate([fc1t[512:1024][a::4] for a in range(4)])
    return {
        "blobf": blobf,
        "blobb": blobb.astype(BF),
        "fc1t": fc1t,
        "fc2t": fc2_w.T.astype(BF).copy(),
    }


_RUN_KWARGS = {}   # test harness can set e.g. {"trace": True}
_LAST_RESULT = None


def kernel(**inputs):
    from concourse.bass_utils import run_bass_kernel_spmd

    k = int(np.asarray(inputs["freq_topk"]))
    if k not in _BUILD_CACHE:
        _BUILD_CACHE[k] = _build(k)
    nc = _BUILD_CACHE[k]

    const = _host_inputs(inputs)
    x = np.ascontiguousarray(np.asarray(inputs["batch_x"], np.float32))
    in_maps = []
    for c in range(NCORES):
        m = dict(const)
        m["x"] = np.ascontiguousarray(x[c * BPC:(c + 1) * BPC])
        in_maps.append(m)

    # occasional transient NRT_EXEC_UNIT_UNRECOVERABLE on this fleet; retry
    last_exc = None
    for attempt in range(3):
        try:
            res = run_bass_kernel_spmd(nc, in_maps,
                                       core_ids=list(range(NCORES)),
                                       **_RUN_KWARGS)
            outs = [np.asarray(res.results[c]["out"]) for c in range(NCORES)]
            globals()["_LAST_RESULT"] = res
            return np.concatenate(outs, axis=1).astype(np.float32)
        except Exception as e:  # noqa: BLE001
            last_exc = e
            import time
            time.sleep(2.0 * (attempt + 1))
    raise last_exc


if __name__ == "__main__":
    d = np.load("/tmp/ref_inputs.npz")
    inputs = {kk: d[kk] for kk in d.files}
    out = kernel(**inputs)
    ref = np.load("/tmp/ref_out.npy")
    rel = np.linalg.norm(out - ref) / np.linalg.norm(ref)
    print("rel err:", rel)
